# revision 37
# baseline (speedup 1.0000x reference)
"""Distributed Bass kernel for a 3-layer GCN (BaseGNN) on 8 TRN2 NeuronCores.

Strategy (see sharding hint): nodes are block-partitioned across the 8 cores
(12500 each); every edge is assigned to the core owning its destination.
The GCN symmetric norm factorizes: norm_e = dinv[src]*dinv[dst], so node
features are pre-scaled by dinv before being shared, aggregation is a pure
unweighted segment-sum, and results are post-scaled by dinv.

conv1 + conv2, per core:
  1. gathers the (pre-scaled, bf16) features of its edges' sources from a
     replicated table via dma_gather (int16 indices -> 4 table chunks),
  2. segment-sums them via one-hot selector matmuls on the TensorEngine
     (messages as lhsT -> aggregate lands feature-major [F, nodes]).
     conv1 runs superblock-major (chunk inner): each 4-block PSUM bank
     accumulates all 4 chunks and drains once, so the downstream transform
     pipelines behind the gather stream. conv2 runs chunk-major, with each
     chunk's gathers gated on that quarter's AllGather.
  3. applies the linear transform + residual + bias + LayerNorm + ReLU in
     feature-major layout (LN stats via ones-vector matmuls),
  4. (conv1 only) re-scales by dinv, transposes to node-major, and
     AllGathers the conv2 table quarter by quarter.

conv3 + mean-pooling are folded into a single host-precomputed matrix:
pooled = C' @ h2 @ W3 + b3, with C'[g,s] = sum_{edges s->d, batch[d]=g}
dinv[s]dinv[d]/cnt[g] (conv3 is linear and pooling commutes with it).
On device that is 98 accumulating [nodes,128f]x[nodes,64g] matmuls over
transposed h2 blocks, a [128,64] AllReduce, and one final W3 matmul.

The edge plan (tile counts per (chunk, dst-block)) is shared across cores
(max over cores) so the single SPMD program fits every core's data.
"""
import sys, os, time, math, tempfile

sys.path.insert(0, "/opt/trn_rl_repo")
import numpy as np
import ml_dtypes

BF = ml_dtypes.bfloat16

# ---------------- problem constants (hardcoded; kernel.py must be standalone)
N, E, B = 100000, 1600000, 64
IN_D, HID, OUT_D = 20, 128, 256
EPS = 1e-5
NCORE = 8
NLOC = N // NCORE            # 12500 real nodes per core
BLK = 128
NBLK = (NLOC + BLK - 1) // BLK       # 98
NLOCP = NBLK * BLK                   # 12544 padded rows per core in tables
NP = NCORE * NLOCP                   # 100352 padded table rows
# gather-table quarters: table laid out [quarter][core][rows-in-quarter] so
# each quarter is produced by its own AllGather and forms one gather chunk
# (rows per chunk <= 25600 < int16 index limit).
GT = int(os.environ.get("KGT", "8"))   # tiles of 128 edges per dma_gather call (1024-idx ucode limit)
SCRATCH = int(os.environ.get("KSCRATCH", str(max(16384, GT * 128 * 16))))
SBW = 4                              # blocks per conv1 superblock (PSUM bank)
LNT = 500                            # nodes per LayerNorm/transform tile
NLNT = NLOC // LNT                   # 25
STG = 7                              # blocks per output staging group
NSTG = NBLK // STG                   # 14

F32 = np.float32


def _quarters():
    """Quarter partition of the per-core blocks (derived from current globals
    so tiny-scale tests can monkeypatch NBLK etc.)."""
    nq = min(4, NBLK)
    qblk = [NBLK // nq + (1 if i < NBLK % nq else 0) for i in range(nq)]
    qb0 = [sum(qblk[:i]) for i in range(nq)]
    qrows = [q * BLK for q in qblk]
    chunkrows = [NCORE * r for r in qrows]
    choff = [sum(chunkrows[:i]) for i in range(nq)]
    return nq, qblk, qb0, qrows, chunkrows, choff


def _table_row(n):
    """real global node id -> row in the quarter-major table layout,
    plus (chunk index, row-within-chunk)."""
    nq, qblk, qb0, qrows, chunkrows, choff = _quarters()
    r = n // NLOC
    l = n % NLOC
    b = l // BLK
    q = np.searchsorted(np.asarray(qb0[1:], np.int64), b, side="right")
    qb0a = np.asarray(qb0, np.int64)[q]
    qra = np.asarray(qrows, np.int64)[q]
    cha = np.asarray(choff, np.int64)[q]
    inchunk = r * qra + (l - qb0a * BLK)
    return cha + inchunk, q, inchunk


def build_plan(edge_index):
    """Per-core edge arrays + common (max-over-cores) tile-count table."""
    NCHUNK = _quarters()[0]
    src = np.concatenate([edge_index[0], np.arange(N, dtype=np.int64)])
    dst = np.concatenate([edge_index[1], np.arange(N, dtype=np.int64)])
    core = dst // NLOC
    per_core = []
    counts = np.zeros((NCORE, NCHUNK, NBLK), np.int64)
    for c in range(NCORE):
        m = core == c
        _, chunk, s = _table_row(src[m])
        d = dst[m] - c * NLOC
        block = d // BLK
        order = np.lexsort((block, chunk))
        s, d, chunk, block = s[order], d[order], chunk[order], block[order]
        key = chunk * NBLK + block
        bounds = np.searchsorted(key, np.arange(NCHUNK * NBLK + 1))
        per_core.append((s, d, bounds))
        counts[c] = (bounds[1:] - bounds[:-1]).reshape(NCHUNK, NBLK)
    ntiles = ((counts.max(0) + 127) // 128).astype(np.int64)      # [NCHUNK, NBLK]
    EP = int(ntiles.sum()) * 128
    NSB = (NBLK + SBW - 1) // SBW

    def cell_orders():
        """yield (ck, b) cells in conv2 (chunk-major) and conv1 (superblock-
        major, chunk-inner) orders."""
        o2 = [(ck, b) for ck in range(NCHUNK) for b in range(NBLK)]
        o1 = [(ck, b)
              for sb in range(NSB)
              for ck in range(NCHUNK)
              for b in range(sb * SBW, min((sb + 1) * SBW, NBLK))]
        return o1, o2

    o1, o2 = cell_orders()
    plans = []
    for c in range(NCORE):
        s, d, bounds = per_core[c]

        def fill(order):
            idx = np.zeros(EP, np.int64)
            dl = np.full(EP, 300, np.int64)
            pos = 0
            for ck, b in order:
                gi = ck * NBLK + b
                lo, hi = bounds[gi], bounds[gi + 1]
                n = hi - lo
                idx[pos:pos + n] = s[lo:hi]
                dl[pos:pos + n] = d[lo:hi] - b * BLK
                pos += int(ntiles[ck, b]) * 128
            # wrap idx: slot i -> [i%16, i//16], replicated x8 on partitions
            w = idx.reshape(-1, 16).T.astype(np.int16)     # [16, EP/16]
            idxw = np.tile(w, (8, 1))                      # [128, EP/16]
            dlw = dl.reshape(-1, 128).T.astype(BF)         # [128, EP/128]
            return np.ascontiguousarray(idxw), np.ascontiguousarray(dlw)

        i1, d1 = fill(o1)
        i2, d2 = fill(o2)
        plans.append({"idx": i2, "dl": d2, "idx1": i1, "dl1": d1})
    return plans, ntiles, EP


# ---------------- device program ----------------------------------------

DEBUG = False


def build_nc(ntiles):
    from concourse import bass, bacc, tile
    from concourse.tile_rust import add_dep_helper
    from concourse.bass import mybir
    from concourse.alu_op_type import AluOpType as op
    f32, bf16, i16 = mybir.dt.float32, mybir.dt.bfloat16, mybir.dt.int16
    AF = mybir.ActivationFunctionType

    EP = int(ntiles.sum()) * 128
    nc = bacc.Bacc("TRN2", target_bir_lowering=False, num_swdge_queues=4,
                   dynamic_dma_scratch_size=SCRATCH)

    xs_d = nc.declare_dram_parameter("xs", [NP, 128], bf16, isOutput=False)
    idx_d = nc.declare_dram_parameter("idx", [128, EP // 16], i16, isOutput=False)
    dl_d = nc.declare_dram_parameter("dl", [128, EP // 128], bf16, isOutput=False)
    idx1_d = nc.declare_dram_parameter("idx1", [128, EP // 16], i16, isOutput=False)
    dl1_d = nc.declare_dram_parameter("dl1", [128, EP // 128], bf16, isOutput=False)
    wts_d = nc.declare_dram_parameter("wts", [128, 640], bf16, isOutput=False)
    wc_d = nc.declare_dram_parameter("wconst", [128, 138], f32, isOutput=False)
    misc_d = nc.declare_dram_parameter("misc", [128, 256], bf16, isOutput=False)
    xloc_d = nc.declare_dram_parameter("xloc", [32, NLOC], bf16, isOutput=False)
    dinvf_d = nc.declare_dram_parameter("dinvf", [1, NLOC], f32, isOutput=False)
    dinvc_d = nc.declare_dram_parameter("dinvc", [128, NBLK], f32, isOutput=False)
    cct_d = nc.declare_dram_parameter("cct", [128, NBLK * 64], bf16, isOutput=False)
    ones_d = nc.declare_dram_parameter("onesrow", [1, 128], f32, isOutput=False)
    b3r_d = nc.declare_dram_parameter("b3row", [1, OUT_D], f32, isOutput=False)
    out_d = nc.declare_dram_parameter("out", [64, OUT_D], f32, isOutput=True)
    if DEBUG:
        dbg_agg1 = nc.declare_dram_parameter("dbg_agg1", [128, NLOC], f32, isOutput=True)
        dbg_h1 = nc.declare_dram_parameter("dbg_h1", [128, NLOC], f32, isOutput=True)
        dbg_agg2 = nc.declare_dram_parameter("dbg_agg2", [128, NLOC], f32, isOutput=True)
        dbg_p1 = nc.declare_dram_parameter("dbg_p1", [NP, 128], f32, isOutput=True)
        dbg_ccp = nc.declare_dram_parameter("dbg_ccp", [NLOCP, 128], f32, isOutput=True)

    # wts columns
    W2c, W3ac, W3bc, W1c, RWc = (slice(0, 128), slice(128, 256), slice(256, 384),
                                 slice(384, 512), slice(512, 640))
    # wconst columns
    BIAS1, G1, B1, B2c, G2, B2l, B3A, B3B, ONES, EPSC = range(10)
    IDF0 = 10  # f32 identity block wc[:, 10:138]

    NQ, QBLK, QB0, QROWS, CHUNKROWS, CHOFF = _quarters()
    NCHUNK = NQ
    MAXCT = int(ntiles.sum(axis=1).max())
    rg = [list(range(NCORE))]

    with tile.TileContext(nc) as tc:
        with (
            tc.tile_pool(name="const", bufs=1) as cpool,
            tc.tile_pool(name="state", bufs=1) as spool,
            tc.tile_pool(name="io", bufs=2) as iopool,
            tc.tile_pool(name="work", bufs=int(os.environ.get("KBUFS", "6"))) as wpool,
            tc.tile_pool(name="ln", bufs=2) as lnpool,
            tc.tile_pool(name="main_ps", bufs=2, space="PSUM") as mainps,
            tc.tile_pool(name="st_ps", bufs=1, space="PSUM") as stps,
            tc.tile_pool(name="aux_ps", bufs=4, space="PSUM") as auxps,
            tc.tile_pool(name="pool_ps", bufs=1, space="PSUM") as poolps,
            tc.tile_pool(name="dram", bufs=1, space="DRAM") as dram,
        ):
            # ---- persistent DRAM scratch
            KREP0 = int(os.environ.get("KREP", "1"))
            PSPACE = os.environ.get("KSHARED", "Local")
            ccp_in1 = [dram.tile([QROWS[q], 128], bf16, name=f"cc1_{q}")
                       for q in range(NQ)]
            p1_fulls = [[dram.tile([CHUNKROWS[q], 128], bf16, addr_space=PSPACE,
                                   name=f"p1f{r}_{q}") for q in range(NQ)]
                        for r in range(KREP0)]
            ccq_ins = [dram.tile([128, 64], f32, name=f"cqi{r}")
                       for r in range(KREP0)]
            ccq_outs = [dram.tile([128, 64], f32, addr_space="Shared",
                                  name=f"cqo{r}")
                        for r in range(KREP0)]

            # ---- constants
            wts = cpool.tile([128, 640], bf16)
            wc = cpool.tile([128, 138], f32)
            misc = cpool.tile([128, 256], bf16)
            dinvc = cpool.tile([128, NBLK], f32)
            onesr = cpool.tile([1, 128], f32)
            b3row = cpool.tile([1, OUT_D], f32)
            cct = cpool.tile([128, NBLK * 64], bf16)
            nc.sync.dma_start(wts[:], wts_d[:])
            nc.sync.dma_start(wc[:], wc_d[:])
            nc.sync.dma_start(misc[:], misc_d[:])
            nc.sync.dma_start(dinvc[:], dinvc_d[:])
            nc.sync.dma_start(onesr[:], ones_d[:])
            nc.sync.dma_start(b3row[:], b3r_d[:])
            nc.sync.dma_start(cct[:], cct_d[:])
            iota = misc[:, 0:128]
            ident = misc[:, 128:256]

            # ---- persistent SBUF state
            aggT = spool.tile([128, NLOC], bf16)
            h1T = spool.tile([128, NLOC], bf16)
            hXT = spool.tile([128, NLOC], bf16)

            pool_ps = poolps.tile([128, 64], f32)

            def aggregate(conv, tables, deps=None):
                """fill aggT[0:Fa, :] with the plain segment-sum.
                tables: per-chunk gather-source APs; deps: per-chunk producer
                instructions (AllGathers) the gathers must wait on."""
                Fa = 32 if conv == 1 else 128
                tile_base = 0
                qrr = [0]
                for ck in range(NCHUNK):
                    tmap = {}
                    ck_tiles = int(ntiles[ck].sum())
                    # whole-chunk index/dl loads (one DMA each)
                    idxt = iopool.tile([128, 8 * MAXCT], i16, tag="idxt")
                    nc.sync.dma_start(
                        idxt[:, : 8 * ck_tiles],
                        idx_d[:, 8 * tile_base: 8 * (tile_base + ck_tiles)])
                    dlt = iopool.tile([128, MAXCT], bf16, tag="dlt")
                    nc.sync.dma_start(dlt[:, :ck_tiles],
                                      dl_d[:, tile_base: tile_base + ck_tiles])
                    # gather calls
                    t0 = 0
                    while t0 < ck_tiles:
                        ntc = min(GT, ck_tiles - t0)
                        msg = wpool.tile([128, GT, 128], bf16, tag="msg")
                        gi_inst = nc.gpsimd.dma_gather(
                            msg[:, :ntc, :],
                            tables[ck],
                            idxt[:, 8 * t0: 8 * (t0 + ntc)],
                            ntc * 128, ntc * 128, 128,
                            queue_num=qrr[0] % 4)
                        qrr[0] += 1
                        if deps is not None:
                            add_dep_helper(gi_inst.ins, deps[ck].ins,
                                           reason="AllGather -> gather table read")
                        sel = wpool.tile([128, GT, 128], bf16, tag="sel")
                        if not KNOSEL:
                            nc.vector.tensor_tensor(
                                sel[:, :ntc, :],
                                dlt[:, t0: t0 + ntc].unsqueeze(2)
                                .broadcast_to([128, ntc, 128]),
                                iota.unsqueeze(1).broadcast_to([128, ntc, 128]),
                                op.is_equal)
                        else:
                            nc.vector.memset(sel[:, 0, 0:1], 0.0)
                        if KNOMM:
                            nc.vector.tensor_copy(selh[:, 0:1], msg[:, 0, 0:1])
                        for j in range(ntc):
                            tmap[t0 + j] = (msg, sel, j)
                        t0 += ntc
                    # bank matmuls: one [128,512] PSUM bank spans 4 blocks.
                    # The ck>0 reload streams the whole bank (start=True sets
                    # has_written everywhere); for ck==0 the bank's first tile
                    # matmul carries start=True and later matmuls overwrite
                    # their columns on first touch (per-element has_written).
                    toff = 0
                    NSB = (NBLK + SBW - 1) // SBW
                    for bank in range(NSB if not KNOMM else 0):
                        b0 = bank * SBW
                        bn = min(SBW, NBLK - b0)
                        wcols = min(BLK * bn, NLOC - b0 * BLK)
                        bank_tiles = int(ntiles[ck, b0:b0 + bn].sum())
                        if bank_tiles == 0:
                            continue
                        ps = mainps.tile([128, 512], f32, tag="main")
                        cs = slice(b0 * BLK, b0 * BLK + wcols)
                        if ck > 0:
                            nc.tensor.matmul(ps[0:Fa, 0:wcols],
                                             ident[0:Fa, 0:Fa],
                                             aggT[0:Fa, cs],
                                             start=True, stop=False,
                                             skip_group_check=True)
                        ti = 0
                        for k in range(bn):
                            nt = int(ntiles[ck, b0 + k])
                            for j in range(nt):
                                m, s, off = tmap[toff + ti]
                                nc.tensor.matmul(
                                    ps[0:Fa, k * BLK: k * BLK + 128],
                                    m[:, off, 0:Fa], s[:, off, :],
                                    start=(ti == 0 and ck == 0),
                                    stop=(ti == bank_tiles - 1),
                                    skip_group_check=True)
                                ti += 1
                        # drain on ACT (idle during aggregation); DVE keeps
                        # only the selector builds
                        nc.scalar.activation(aggT[0:Fa, cs], ps[0:Fa, 0:wcols],
                                             AF.Identity)
                        toff += bank_tiles
                    tile_base += ck_tiles

            def aggregate1(tables):
                """conv1 segment-sum in superblock-major order (chunk inner):
                each 4-block superblock accumulates all 4 chunks in one PSUM
                bank and drains once, so aggT becomes valid progressively and
                transform/emit/AllGather pipeline behind the gather stream."""
                Fa = 32
                qrr = [0]
                NSB = (NBLK + SBW - 1) // SBW
                MAXSBT = int(max(ntiles[:, sb * SBW: (sb + 1) * SBW].sum()
                                 for sb in range(NSB)))
                tile_base = 0
                for sb in range(NSB):
                    b0 = sb * SBW
                    bn = min(SBW, NBLK - b0)
                    sb_tiles = int(ntiles[:, b0:b0 + bn].sum())
                    sbw_cols = min(BLK * bn, NLOC - b0 * BLK)
                    idxt = iopool.tile([128, 8 * MAXSBT], i16, tag="idxt1")
                    nc.sync.dma_start(
                        idxt[:, : 8 * sb_tiles],
                        idx1_d[:, 8 * tile_base: 8 * (tile_base + sb_tiles)])
                    dlt = iopool.tile([128, MAXSBT], bf16, tag="dlt1")
                    nc.sync.dma_start(dlt[:, :sb_tiles],
                                      dl1_d[:, tile_base: tile_base + sb_tiles])
                    ps = mainps.tile([128, 512], f32, tag="main")
                    # tile -> (block-in-sb) col offset map in conv1 order.
                    # start=True only on the superblock's FIRST matmul: it
                    # clears has_written for the whole bank; later matmuls
                    # (start=False) overwrite-on-first-touch per element, so
                    # per-block chains need no separate start flags.
                    owner = []
                    for ck in range(NCHUNK):
                        for k in range(bn):
                            owner += [k] * int(ntiles[ck, b0 + k])
                    t0 = 0
                    for ck in range(NCHUNK):
                        run = int(ntiles[ck, b0:b0 + bn].sum())
                        t = 0
                        while t < run:
                            ntc = min(GT, run - t)
                            msg = wpool.tile([128, GT, 128], bf16, tag="msg")
                            nc.gpsimd.dma_gather(
                                msg[:, :ntc, :],
                                tables[ck],
                                idxt[:, 8 * (t0 + t): 8 * (t0 + t + ntc)],
                                ntc * 128, ntc * 128, 128,
                                queue_num=qrr[0] % 4)
                            qrr[0] += 1
                            sel = wpool.tile([128, GT, 128], bf16, tag="sel")
                            nc.vector.tensor_tensor(
                                sel[:, :ntc, :],
                                dlt[:, t0 + t: t0 + t + ntc].unsqueeze(2)
                                .broadcast_to([128, ntc, 128]),
                                iota.unsqueeze(1).broadcast_to([128, ntc, 128]),
                                op.is_equal)
                            for j in range(ntc):
                                ti = t0 + t + j      # tile index within sb
                                k = owner[ti]
                                nc.tensor.matmul(
                                    ps[0:Fa, k * BLK: k * BLK + 128],
                                    msg[:, j, 0:Fa], sel[:, j, :],
                                    start=(ti == 0),
                                    stop=(ti == sb_tiles - 1),
                                    skip_group_check=True)
                            t += ntc
                        t0 += run
                    nc.scalar.activation(
                        aggT[0:Fa, b0 * BLK: b0 * BLK + sbw_cols],
                        ps[0:Fa, 0:sbw_cols], AF.Identity)
                    tile_base += sb_tiles

            def transform_ln(conv):
                """aggT -> (transform + bias + residual + LN + relu) -> pT"""
                Fa = 32 if conv == 1 else 128
                for i in range(NLNT):
                    sl = slice(i * LNT, (i + 1) * LNT)
                    dfv = lnpool.tile([1, LNT], f32, tag="dfv")
                    nc.sync.dma_start(dfv[:], dinvf_d[0:1, sl])
                    dbc = auxps.tile([128, LNT], f32, tag="aux")
                    nc.tensor.matmul(dbc[0:Fa, :], onesr[:, 0:Fa], dfv[:],
                                     start=True, stop=True)
                    z = lnpool.tile([128, LNT], bf16, tag="z")
                    nc.vector.tensor_tensor(z[0:Fa, :], aggT[0:Fa, sl],
                                            dbc[0:Fa, :], op.mult)
                    ps = mainps.tile([128, LNT], f32, tag="main", padded_shape=[128, LNT])
                    if conv == 1:
                        xsl = lnpool.tile([32, LNT], bf16, tag="xsl")
                        nc.sync.dma_start(xsl[:], xloc_d[:, sl])
                        nc.tensor.matmul(ps[:], wts[0:32, W1c], z[0:32, :],
                                         start=True, stop=False)
                        nc.tensor.matmul(ps[:], wts[0:32, RWc], xsl[:],
                                         start=False, stop=True)
                    else:
                        nc.tensor.matmul(ps[:], wts[:, W2c], z[:],
                                         start=True, stop=False)
                        nc.tensor.matmul(ps[:], ident, h1T[:, sl],
                                         start=False, stop=True)
                    y = lnpool.tile([128, LNT], f32, tag="y")
                    bcol = wc[:, BIAS1:BIAS1 + 1] if conv == 1 else wc[:, B2c:B2c + 1]
                    nc.scalar.activation(y[:], ps[:], AF.Identity, bias=bcol)
                    y2 = lnpool.tile([128, LNT], f32, tag="y2")
                    nc.scalar.activation(y2[:], y[:], AF.Square)
                    st = stps.tile([64, LNT], f32, tag="st")
                    nc.tensor.matmul(st[0:1, :], wc[:, ONES:ONES + 1], y[:],
                                     start=True, stop=True)
                    nc.tensor.matmul(st[32:33, :], wc[:, ONES:ONES + 1], y2[:],
                                     start=True, stop=True)
                    mu = lnpool.tile([1, LNT], f32, tag="mu")
                    nc.vector.tensor_scalar(mu[:], st[0:1, :], 1.0 / 128, None, op.mult)
                    m2 = lnpool.tile([1, LNT], f32, tag="m2")
                    nc.vector.tensor_tensor(m2[:], mu[:], mu[:], op.mult)
                    var = lnpool.tile([1, LNT], f32, tag="var")
                    nc.vector.scalar_tensor_tensor(var[:], st[32:33, :], 1.0 / 128,
                                                   m2[:], op.mult, op.subtract)
                    sd = lnpool.tile([1, LNT], f32, tag="sd")
                    nc.scalar.activation(sd[:], var[:], AF.Sqrt, bias=wc[0:1, EPSC:EPSC + 1])
                    rstd = lnpool.tile([1, LNT], f32, tag="rstd")
                    nc.vector.reciprocal(rstd[:], sd[:])
                    mr = lnpool.tile([1, LNT], f32, tag="mr")
                    nc.vector.tensor_tensor(mr[:], mu[:], rstd[:], op.mult)
                    bc1 = auxps.tile([128, LNT], f32, tag="aux")
                    nc.tensor.matmul(bc1[:], onesr[:], rstd[:], start=True, stop=True)
                    bc2 = auxps.tile([128, LNT], f32, tag="aux")
                    nc.tensor.matmul(bc2[:], onesr[:], mr[:], start=True, stop=True)
                    xc = lnpool.tile([128, LNT], f32, tag="xc")
                    nc.vector.tensor_tensor(xc[:], y[:], bc1[:], op.mult)
                    xn = lnpool.tile([128, LNT], f32, tag="xn")
                    nc.vector.tensor_tensor(xn[:], xc[:], bc2[:], op.subtract)
                    gcol = wc[:, G1:G1 + 1] if conv == 1 else wc[:, G2:G2 + 1]
                    lcol = wc[:, B1:B1 + 1] if conv == 1 else wc[:, B2l:B2l + 1]
                    hdst = h1T if conv == 1 else hXT
                    nc.scalar.activation(hdst[:, sl], xn[:], AF.Relu,
                                         bias=lcol, scale=gcol)

            def emit_p(hsrc, ccp_in, p_fulls_q):
                """Per table quarter: transpose h node-major, scale by dinv,
                stage, DMA to the quarter bounce, AllGather that quarter.
                Returns the per-quarter AllGather instructions."""
                ags = []
                for q in range(NQ):
                    nb = QBLK[q]
                    g0 = 0
                    while g0 < nb:
                        gsz = min(5, nb - g0)
                        stage = wpool.tile([128, 5, 128], bf16, tag="stage")
                        for k in range(gsz):
                            b = QB0[q] + g0 + k
                            w = min(BLK, NLOC - b * BLK)
                            tp = auxps.tile([128, 128], bf16, tag="aux")
                            nc.tensor.transpose(
                                tp[0:w, :], hsrc[:, b * BLK:b * BLK + w], ident)
                            if w < BLK:
                                nc.vector.memset(stage[:, k, :], 0.0)
                            nc.vector.tensor_scalar(
                                stage[0:w, k, :], tp[0:w, :],
                                dinvc[0:w, b:b + 1], None, op.mult)
                        nc.sync.dma_start(
                            ccp_in[q][g0 * BLK:(g0 + gsz) * BLK, :]
                            .rearrange("(k p) f -> p k f", p=128),
                            stage[:, 0:gsz, :])
                        g0 += gsz
                    ags.append(nc.gpsimd.collective_compute(
                        "AllGather", op.bypass, replica_groups=rg,
                        ins=[ccp_in[q].opt()], outs=[p_fulls_q[q].opt()]))
                return ags

            STOP = int(os.environ.get("KSTOP", "9"))
            KREP = int(os.environ.get("KREP", "1"))
            KNOSEL = bool(int(os.environ.get("KNOSEL", "0")))
            KNOMM = bool(int(os.environ.get("KNOMM", "0")))
            selh = spool.tile([128, 1], bf16)

            def bail():
                nc.gpsimd.dma_start(out_d[0:32, :], aggT[0:32, 0:OUT_D])

            # =================== conv1 ===================
            for _rep in range(KREP):
                xs_tables = [xs_d[CHOFF[q]: CHOFF[q] + CHUNKROWS[q], :]
                         for q in range(NQ)]
            aggregate1(xs_tables)
            if STOP <= 1:
                bail()
                return nc
            if DEBUG:
                nc.sync.dma_start(dbg_agg1[0:32, :], aggT[0:32, :])
            for _rep in range(KREP):
                transform_ln(1)
            if DEBUG:
                nc.gpsimd.dma_start(dbg_h1[:], h1T[:])
            for _rep in range(KREP):
                ag1 = emit_p(h1T, ccp_in1, p1_fulls[_rep])
            if STOP <= 3:
                bail()
                return nc

            # =================== conv2 ===================
            for _rep in range(KREP):
                aggregate(2, [t[:] for t in p1_fulls[_rep]], deps=ag1)
            if STOP <= 4:
                bail()
                return nc
            if DEBUG:
                nc.sync.dma_start(dbg_agg2[:], aggT[:])
            for _rep in range(KREP):
                transform_ln(2)
            if STOP <= 5:
                bail()
                return nc
            # ====== pooling: pooled = (C' @ h2) @ W3 + b3 (conv3 folded into
            # the host-side C' matrix; see kernel()) ======
            for _rep in range(KREP):
                for b in range(NBLK):
                    w = min(BLK, NLOC - b * BLK)
                    bs = slice(b * BLK, b * BLK + w)
                    tp = auxps.tile([128, 128], bf16, tag="aux")
                    nc.tensor.transpose(tp[0:w, :], hXT[:, bs], ident)
                    h2nm = lnpool.tile([128, 128], bf16, tag="h2nm")
                    nc.scalar.activation(h2nm[0:w, :], tp[0:w, :], AF.Identity)
                    nc.tensor.matmul(pool_ps[:],
                                     h2nm[0:w, :],
                                     cct[0:w, b * 64:(b + 1) * 64],
                                     start=(b == 0), stop=(b == NBLK - 1))
                pool_sb = cpool.tile([128, 64], f32, tag=f"pool_sb{_rep}")
                nc.vector.tensor_copy(pool_sb[:], pool_ps[:])
                nc.sync.dma_start(ccq_ins[_rep][:], pool_sb[:])
                nc.gpsimd.collective_compute(
                    "AllReduce", op.add, replica_groups=rg,
                    ins=[ccq_ins[_rep].opt()], outs=[ccq_outs[_rep].opt()])
                par = cpool.tile([128, 64], f32, tag=f"par{_rep}")
                nc.sync.dma_start(par[:], ccq_outs[_rep][:])
                parb = cpool.tile([128, 64], bf16, tag=f"parb{_rep}")
                nc.vector.tensor_copy(parb[:], par[:])
                out_ps = auxps.tile([64, OUT_D], f32, tag="aux")
                nc.tensor.matmul(out_ps[:], parb[:, 0:64], wts[:, 128:384],
                                 start=True, stop=False)
                nc.tensor.matmul(out_ps[:], onesr[0:1, 0:64], b3row[:],
                                 start=False, stop=True)
                osb = cpool.tile([64, OUT_D], f32, tag=f"osb{_rep}")
                nc.vector.tensor_copy(osb[:], out_ps[:])
                nc.sync.dma_start(out_d[:], osb[:])
    return nc


# ---------------- host wrapper -------------------------------------------

_CACHE = {}
_last_in_maps = None


def kernel(x, edge_index, batch, W1, b1, W2, b2, W3, b3, res_W, res_b,
           ln1_g, ln1_b, ln2_g, ln2_b):
    from concourse.bass_utils import run_bass_kernel_spmd

    x = np.asarray(x, F32)
    edge_index = np.asarray(edge_index).astype(np.int64)
    batch = np.asarray(batch).astype(np.int64)

    deg = np.bincount(
        np.concatenate([edge_index[1], np.arange(N, dtype=np.int64)]),
        minlength=N).astype(F32)
    dinv = (1.0 / np.sqrt(deg)).astype(F32)

    plans, ntiles, EP = build_plan(edge_index)

    # pooling matrix: pooled = C' @ h2 @ W3 + b3 with
    # C'[g, s] = sum_{edges s->d (incl self-loops), batch[d]=g} dinv[s]dinv[d]
    #            / cnt[g]
    asrc = np.concatenate([edge_index[0], np.arange(N, dtype=np.int64)])
    adst = np.concatenate([edge_index[1], np.arange(N, dtype=np.int64)])
    cnt = np.bincount(batch, minlength=B).astype(F32)
    wgt = (dinv[asrc] * dinv[adst]).astype(np.float64)
    key = batch[adst] * N + asrc
    Cp = np.bincount(key, weights=wgt, minlength=B * N).reshape(B, N).astype(F32)
    Cp /= np.maximum(cnt, 1.0)[:, None]

    # conv1 table: x*dinv padded into [NP, 128] bf16
    xs = np.zeros((NP, 128), F32)
    rows, _, _ = _table_row(np.arange(N, dtype=np.int64))
    xs[rows, :IN_D] = x * dinv[:, None]
    xs = xs.astype(BF)

    # weights
    wts = np.zeros((128, 640), F32)
    wts[:, 0:128] = W2
    wts[:, 128:256] = W3[:, 0:128]
    wts[:, 256:384] = W3[:, 128:256]
    wts[:IN_D, 384:512] = W1
    wts[:IN_D, 512:640] = res_W
    wts = wts.astype(BF)

    wc = np.zeros((128, 138), F32)
    wc[:, 0] = b1 + res_b
    wc[:, 1], wc[:, 2] = ln1_g, ln1_b
    wc[:, 3], wc[:, 4], wc[:, 5] = b2, ln2_g, ln2_b
    wc[:, 6], wc[:, 7] = b3[0:128], b3[128:256]
    wc[:, 8] = 1.0
    wc[:, 9] = EPS
    wc[:, 10:138] = np.eye(128, dtype=F32)

    misc = np.zeros((128, 256), F32)
    misc[:, 0:128] = np.arange(128, dtype=F32)[None, :]
    misc[:, 128:256] = np.eye(128, dtype=F32)
    misc = misc.astype(BF)

    b3row = np.ascontiguousarray(b3.reshape(1, OUT_D)).astype(F32)

    in_maps = []
    for c in range(NCORE):
        nsl = slice(c * NLOC, (c + 1) * NLOC)
        xloc = np.zeros((32, NLOC), F32)
        xloc[:IN_D] = x[nsl].T
        dtmp = np.zeros(NLOCP, F32)
        dtmp[:NLOC] = dinv[nsl]
        dinvc = np.ascontiguousarray(dtmp.reshape(NBLK, BLK).T)
        # cct[p, b*64+g] = C'[g, node(c, b*128+p)]  (node-major wrap)
        ctmp = np.zeros((NLOCP, 64), F32)
        ctmp[:NLOC] = Cp[:, nsl].T
        cct = np.ascontiguousarray(
            ctmp.reshape(NBLK, BLK, 64).transpose(1, 0, 2).reshape(128, NBLK * 64)).astype(BF)
        in_maps.append({
            "xs": xs, "idx": plans[c]["idx"], "dl": plans[c]["dl"],
            "idx1": plans[c]["idx1"], "dl1": plans[c]["dl1"],
            "wts": wts, "wconst": wc, "misc": misc,
            "xloc": xloc.astype(BF),
            "dinvf": np.ascontiguousarray(dinv[nsl]).reshape(1, NLOC),
            "dinvc": dinvc,
            "cct": cct, "b3row": b3row,
            "onesrow": np.ones((1, 128), F32),
        })

    global _last_in_maps
    _last_in_maps = in_maps
    key = (os.environ.get("KSTOP", "9"), GT, ntiles.tobytes())
    if key not in _CACHE:
        t0 = time.time()
        nc = build_nc(ntiles)
        print(f"[kernel] traced in {time.time()-t0:.1f}s", file=sys.stderr)
        t0 = time.time()
        nc.compile()
        print(f"[kernel] bacc-compiled in {time.time()-t0:.1f}s", file=sys.stderr)
        _CACHE[key] = nc
    nc = _CACHE[key]

    t0 = time.time()
    trace = bool(int(os.environ.get("KTRACE", "0")))
    res = run_bass_kernel_spmd(nc, in_maps, core_ids=list(range(NCORE)),
                               trace=trace)
    print(f"[kernel] ran in {time.time()-t0:.1f}s", file=sys.stderr)
    kernel.last_results = res
    return np.asarray(res.results[0]["out"], F32)

